# revision 17
# baseline (speedup 1.0000x reference)
"""GCN message-passing kernel for Trainium2 (8 NeuronCores, Bass/Tile).

out = coef * relu(C_U * D^-1/2 A~^T D^-1/2 (x W^T + b)),  A~ = A + I

Strategy (dst-sharded, fully static SPMD program, host-materialized
message stream):
- Core c owns a 12,500-node dst range. Host assigns dsts to 98
  windows of up to 128 members (LPT by in-degree for load balance),
  then lays every edge message (incl. self loops) out window-major as
  a dense fp16 stream xg[lane, col, :] = dis_src * x[src]  (the
  degree prescale and fp16 cast are done once on host; W commutes
  with aggregation so it is applied after the scatter).
- Device: pure sequential DMA of the xg stream (no gather); DVE
  builds one-hot [slot,128] matrices from dst offsets (iota==doff);
  PE contracts msgs^T @ onehot into PSUM [128=D, 128=dst] per window;
  stage-2 matmul applies W^T; ACT fuses relu + coef*C_U*dis_dst
  scale; fp16 DMA out (host upcasts + unpermutes).
"""

import sys
import types

import numpy as np


def _install_ntff_hook_bridge():
    """antenv.axon_hooks is missing from this image; bridge it so
    run_bass_kernel_spmd(trace=True) can profile. Harmless if unused."""
    if "antenv.axon_hooks" in sys.modules:
        return
    hooks = types.ModuleType("antenv.axon_hooks")
    hooks._HOOK = None

    def _get():
        if hooks._HOOK is None:
            try:
                from trn_agent_boot.trn_boot import _ntff_profile_via_ctypes

                hooks._HOOK = _ntff_profile_via_ctypes("/opt/axon/libaxon_pjrt.so")
            except Exception:
                hooks._HOOK = None
        return hooks._HOOK

    hooks.get_axon_ntff_profile_hook = _get
    hooks.set_axon_ntff_profile_hook = lambda h: setattr(hooks, "_HOOK", h)
    sys.modules["antenv.axon_hooks"] = hooks


_install_ntff_hook_bridge()

C_SIGMA = 2.0
C_U = 1.0
W_WIN = 64  # dst window width (one-hot width)
N_CORES = 8
GMAX_COLS = 114  # max slot-columns per double-buffered group (~3.7MB DMA)


def _ceil(a, b):
    return (a + b - 1) // b


class _Prep:
    """Host-side sharding/preprocessing result."""


def _pack_windows(indeg, nwin, cap):
    """LPT: assign items (sorted by weight desc) to nwin bins with
    member cap; then swap-rebalance toward max load <= ceil128(mean).
    Returns (win_of, pos_of, loads, counts)."""
    n = indeg.shape[0]
    order = np.argsort(-indeg, kind="stable")
    loads = np.zeros(nwin, dtype=np.int64)
    counts = np.zeros(nwin, dtype=np.int64)
    win_of = np.empty(n, dtype=np.int32)
    big = np.int64(1) << 50
    for d in order:
        w = int(np.argmin(loads))
        win_of[d] = w
        counts[w] += 1
        loads[w] += indeg[d]
        if counts[w] >= cap:
            loads[w] += big  # full -> never chosen again
    loads[loads >= big] -= big
    # swap rebalance: move load between windows via member swaps (keeps
    # member counts) until each window fits its per-window capacity.
    # Two-tier capacities concentrate the above-mean remainder in a few
    # overflow windows instead of pushing every window over a column
    # boundary.
    S = int(indeg.sum())
    base = _ceil(_ceil(S, nwin), 128) * 128
    caps = np.full(nwin, base, dtype=np.int64)
    base2 = base - 128
    excess2 = S - nwin * base2
    n_over = _ceil(max(excess2, 0), 128)
    if 0 < n_over < nwin:
        caps[:] = base2
        heavy = np.argsort(-loads)[:n_over]
        caps[heavy] = base2 + 128 * _ceil(_ceil(excess2, n_over), 128)
    for _ in range(3000):
        viol = loads - caps
        hi = int(np.argmax(viol))
        if viol[hi] <= 0:
            break
        lo = int(np.argmin(viol))
        mh = np.where(win_of == hi)[0]
        ml = np.where(win_of == lo)[0]
        da, db = indeg[mh], indeg[ml]
        diff = da[:, None] - db[None, :]
        ok = diff > 0
        if not ok.any():
            break
        head = -viol[lo]
        tgt = min(viol[hi], head) if head > 0 else 1
        score = np.where(ok, np.abs(diff - tgt), np.int64(1) << 40)
        i, j = np.unravel_index(np.argmin(score), score.shape)
        a, b = mh[i], ml[j]
        win_of[a], win_of[b] = lo, hi
        delta = indeg[a] - indeg[b]
        loads[hi] -= delta
        loads[lo] += delta
    # positions: recompute per window
    pos_of = np.empty(n, dtype=np.int32)
    counts = np.zeros(nwin, dtype=np.int64)
    for d in range(n):
        w = win_of[d]
        pos_of[d] = counts[w]
        counts[w] += 1
    return win_of, pos_of, loads, counts


def prepare(x, edge_index, W, b, n_cores=N_CORES, w_win=W_WIN):
    import ml_dtypes

    f16 = ml_dtypes.bfloat16
    N, D = x.shape
    assert N % n_cores == 0
    npc = N // n_cores
    nwin = _ceil(npc, w_win)
    if nwin % 2:
        nwin += 1  # window pairing needs an even count

    src = np.asarray(edge_index[0], dtype=np.int64)
    dst = np.asarray(edge_index[1], dtype=np.int64)
    deg = np.bincount(src, minlength=N).astype(np.float32) + 1.0
    dis = deg ** -0.5  # float32

    p = _Prep()
    p.N, p.D, p.npc, p.nwin = N, D, npc, nwin
    p.n_cores, p.w_win = n_cores, w_win
    p.coef = np.sqrt(C_SIGMA / D).astype(np.float32)
    p.xs16 = (dis[:, None] * np.asarray(x, dtype=np.float32)).astype(f16)

    core_of = dst // npc
    dstloc = (dst - core_of * npc).astype(np.int64)

    # --- per-core window assignment (LPT by in-degree incl self loop)
    p.win_of = np.empty((n_cores, npc), dtype=np.int32)
    p.pos_of = np.empty((n_cores, npc), dtype=np.int32)
    loads_all = np.empty((n_cores, nwin), dtype=np.int64)
    for c in range(n_cores):
        indeg = np.bincount(dstloc[core_of == c], minlength=npc) + 1
        win_of, pos_of, loads, _ = _pack_windows(indeg, nwin, w_win)
        # canonical order: windows sorted by load desc so the static
        # per-window column schedule (max over cores) stays tight
        order = np.argsort(-loads, kind="stable")
        rank = np.empty(nwin, dtype=np.int32)
        rank[order] = np.arange(nwin, dtype=np.int32)
        p.win_of[c] = rank[win_of]
        p.pos_of[c] = pos_of
        loads_all[c] = loads[order]

    # --- static per-window column schedule (shared by all cores)
    p.wcols = np.maximum(1, _ceil(loads_all.max(axis=0), 128)).astype(np.int64)
    p.colstart = np.concatenate([[0], np.cumsum(p.wcols)[:-1]])
    p.totcols = int(p.wcols.sum())

    # --- groups of consecutive windows for double-buffered streaming
    # (even window counts preferred so stage-1 chains interleave in pairs)
    p.groups = []  # (w0, nwins, colstart, ncols)
    w0 = 0
    while w0 < nwin:
        w1 = w0
        cols = 0
        # small head group (compute starts sooner) and small tail groups
        # (less un-overlapped compute after the last load)
        if w0 == 0 or nwin - w0 <= 8:
            gcap = 4 * int(p.wcols[0])
        else:
            gcap = GMAX_COLS
        while w1 < nwin and cols + p.wcols[w1] <= gcap:
            cols += int(p.wcols[w1])
            w1 += 1
        if (w1 - w0) > 1 and (w1 - w0) % 2 == 1 and w1 < nwin:
            cols -= int(p.wcols[w1 - 1])
            w1 -= 1
        p.groups.append((w0, w1 - w0, int(p.colstart[w0]), cols))
        w0 = w1
    p.gmax_cols = max(g[3] for g in p.groups)
    p.gmax_wins = max(g[1] for g in p.groups)

    # --- per-core edge slot fill
    p.xg = []
    p.doff = []
    p.sd = []
    p.memb = []
    for c in range(n_cores):
        m = core_of == c
        e_src = np.concatenate([src[m], np.arange(npc, dtype=np.int64) + c * npc])
        e_dl = np.concatenate([dstloc[m], np.arange(npc, dtype=np.int64)])
        e_w = p.win_of[c][e_dl]
        e_off = p.pos_of[c][e_dl]
        o = np.argsort(e_w, kind="stable")
        e_src, e_w, e_off = e_src[o], e_w[o], e_off[o]
        wcnt = np.bincount(e_w, minlength=nwin)
        starts = np.concatenate([[0], np.cumsum(wcnt)[:-1]])
        rank = np.arange(len(e_w)) - starts[e_w]
        lane = rank % 128
        col = p.colstart[e_w] + rank // 128
        assert (rank < p.wcols[e_w] * 128).all()

        xg = np.zeros((128, p.totcols, D), dtype=f16)
        xg[lane, col] = p.xs16[e_src]
        doff = np.full((128, p.totcols), -1.0, dtype=f16)
        doff[lane, col] = e_off
        p.xg.append(xg)
        p.doff.append(doff)

        sd = np.zeros((w_win, nwin), dtype=np.float32)
        sd[p.pos_of[c], p.win_of[c]] = p.coef * C_U * dis[c * npc : (c + 1) * npc]
        # pair windows (2q, 2q+1) stacked on 128 partitions
        sdp = np.zeros((2 * w_win, nwin // 2), dtype=np.float32)
        sdp[:w_win] = sd[:, 0::2]
        sdp[w_win:] = sd[:, 1::2]
        p.sd.append(sdp)
        memb = -np.ones(nwin * w_win, dtype=np.int64)
        memb[p.win_of[c].astype(np.int64) * w_win + p.pos_of[c]] = np.arange(npc)
        p.memb.append(memb)

    # iota row constant [128, 1, w_win]: value j at (p, 0, j)
    io = np.broadcast_to(
        np.arange(w_win, dtype=np.float32)[None, None, :], (128, 1, w_win)
    )
    p.iota = np.ascontiguousarray(io.astype(f16))
    p.WT = np.ascontiguousarray(np.asarray(W, dtype=np.float32).T)
    p.b = np.asarray(b, dtype=np.float32)
    p.bias_nonzero = bool(np.any(p.b != 0))
    if p.bias_nonzero:
        # S_d = sum over incoming edges (incl self) of dis_src, laid out
        # as a row [1, nwin*w_win]: element w*w_win + i = window w, row i
        sb = np.zeros((n_cores, nwin * w_win), dtype=np.float32)
        slot = (
            p.win_of[core_of, dstloc].astype(np.int64) * w_win
            + p.pos_of[core_of, dstloc]
        )
        np.add.at(sb, (core_of, slot), dis[src])
        sl = np.arange(N, dtype=np.int64)
        slot_self = (
            p.win_of[core_of_sl := sl // npc, sl % npc].astype(np.int64) * w_win
            + p.pos_of[core_of_sl, sl % npc]
        )
        np.add.at(sb, (core_of_sl, slot_self), dis[sl])
        p.sb = sb.reshape(n_cores, 1, nwin * w_win)
    return p


def build_program(p):
    import concourse.bacc as bacc
    import concourse.mybir as mybir
    import concourse.tile as tile

    f32, f16i = mybir.dt.float32, mybir.dt.bfloat16
    D, nwin, w_win = p.D, p.nwin, p.w_win

    nc = bacc.Bacc("TRN2", target_bir_lowering=False, debug=False)
    xg_d = nc.dram_tensor("xg", [128, p.totcols, D], f16i, kind="ExternalInput")
    wt_d = nc.dram_tensor("wt", [D, D], f32, kind="ExternalInput")
    iota_d = nc.dram_tensor("iota", [128, 1, w_win], f16i, kind="ExternalInput")
    doff_d = nc.dram_tensor("doff", [128, p.totcols], f16i, kind="ExternalInput")
    sd_d = nc.dram_tensor("sd", [2 * w_win, nwin // 2], f32, kind="ExternalInput")
    if p.bias_nonzero:
        sb_d = nc.dram_tensor("sb", [1, nwin * w_win], f32, kind="ExternalInput")
        b_d = nc.dram_tensor("b", [1, D], f32, kind="ExternalInput")
    # partition-major layout: out[r*64+i, q, :] = window 2q+r, row i
    out_d = nc.dram_tensor("out", [2 * w_win, nwin // 2, D], f16i, kind="ExternalOutput")

    with tile.TileContext(nc) as tc:
        with (
            tc.tile_pool(name="const", bufs=1) as constp,
            tc.tile_pool(name="msgs", bufs=3) as msgsp,
            tc.tile_pool(name="vh", bufs=3) as vhp,
            tc.tile_pool(name="aggx", bufs=3) as aggxp,
            tc.tile_pool(name="outsb", bufs=2) as outp,
            tc.tile_pool(name="ps1", bufs=4, space="PSUM") as ps1p,
            tc.tile_pool(name="ps2", bufs=2, space="PSUM") as ps2p,
        ):
            # constants
            wt32 = constp.tile([D, D], f32, tag="wt32")
            nc.sync.dma_start(wt32[:], wt_d[:])
            wt16 = constp.tile([D, D], f16i, tag="wt16")
            nc.scalar.copy(wt16[:], wt32[:])
            iota_sb = constp.tile([128, 1, w_win], f16i, tag="iota")
            nc.sync.dma_start(iota_sb[:], iota_d[:])
            sd_sb = constp.tile([2 * w_win, nwin // 2], f32, tag="sd")
            nc.sync.dma_start(sd_sb[:], sd_d[:])
            doff_sb = constp.tile([128, p.totcols], f16i, tag="doff")
            nc.sync.dma_start(doff_sb[:], doff_d[:])
            if p.bias_nonzero:
                sb_sb = constp.tile([1, nwin * w_win], f32, tag="sb")
                nc.sync.dma_start(sb_sb[:], sb_d[:])
                b32 = constp.tile([1, D], f32, tag="b32")
                nc.sync.dma_start(b32[:], b_d[:])
                b16 = constp.tile([1, D], f16i, tag="b16")
                nc.scalar.copy(b16[:], b32[:])
                sbrow16 = constp.tile([1, nwin * w_win], f16i, tag="sbw16")
                nc.scalar.copy(sbrow16[:], sb_sb[:])

            for w0, gs, c0, gcols in p.groups:
                ms = msgsp.tile([128, gcols, D], f16i, tag="ms")
                nc.sync.dma_start(ms[:], xg_d[:, c0 : c0 + gcols, :])
                vt = vhp.tile([128, gcols, w_win], f16i, tag="vh")
                nc.vector.tensor_tensor(
                    vt[:],
                    iota_sb[:].broadcast_to([128, gcols, w_win]),
                    doff_sb[:, c0 : c0 + gcols]
                    .rearrange("p (c o) -> p c o", o=1)
                    .broadcast_to([128, gcols, w_win]),
                    mybir.AluOpType.is_equal,
                )
                out_sb = outp.tile([2 * w_win, gs // 2, D], f16i, tag="out")
                # windows processed in pairs sharing one PSUM bank: chain r
                # accumulates into ps1[:, r*64:(r+1)*64]; one ag copy,
                # stage-2 matmul and relu+scale activation per pair
                for pl0 in range(0, gs, 2):
                    ps1 = ps1p.tile([D, 2 * w_win], f32, tag="ps1")
                    for r in (0, 1):
                        w = w0 + pl0 + r
                        cw = int(p.wcols[w])
                        cl = int(p.colstart[w]) - c0
                        for k in range(cw):
                            nc.tensor.matmul(
                                ps1[:, r * w_win : (r + 1) * w_win],
                                ms[:, cl + k, :],
                                vt[:, cl + k, :],
                                start=(k == 0),
                                stop=(k == cw - 1),
                            )
                    ag = aggxp.tile([D, 2 * w_win], f16i, tag="ag")
                    nc.scalar.copy(ag[:], ps1[:])
                    ps2 = ps2p.tile([2 * w_win, D], f32, tag="ps2")
                    nc.tensor.matmul(
                        ps2[:, :],
                        ag[:, :],
                        wt16[:, :],
                        start=True,
                        stop=not p.bias_nonzero,
                    )
                    q = (w0 + pl0) // 2
                    if p.bias_nonzero:
                        nc.tensor.matmul(
                            ps2[:, :],
                            sbrow16[:, q * 2 * w_win : (q + 1) * 2 * w_win],
                            b16[:, :],
                            start=False,
                            stop=True,
                        )
                    nc.scalar.activation(
                        out_sb[:, pl0 // 2, :],
                        ps2[:, :],
                        mybir.ActivationFunctionType.Relu,
                        scale=sd_sb[:, q : q + 1],
                    )
                nc.scalar.dma_start(out_d[:, w0 // 2 : (w0 + gs) // 2, :], out_sb[:])
    nc.compile()
    return nc


def make_in_maps(p):
    in_maps = []
    for c in range(p.n_cores):
        m = {
            "xg": p.xg[c],
            "wt": p.WT,
            "iota": p.iota,
            "doff": p.doff[c],
            "sd": p.sd[c],
        }
        if p.bias_nonzero:
            m["sb"] = p.sb[c]
            m["b"] = p.b.reshape(1, -1)
        in_maps.append(m)
    return in_maps


def _unshard(p, outs):
    N, D = p.N, p.D
    res = np.empty((N, D), dtype=np.float32)
    for c in range(p.n_cores):
        # [2*w_win, nwin//2, D]: window 2q+r at rows [r*w_win:(r+1)*w_win]
        o = np.asarray(outs[c]).astype(np.float32)
        o = (
            o.reshape(2, p.w_win, p.nwin // 2, D)
            .transpose(2, 0, 1, 3)
            .reshape(p.nwin * p.w_win, D)
        )
        memb = p.memb[c]
        real = memb >= 0
        res[c * p.npc + memb[real]] = o[real]
    return res


def kernel(x, edge_index, W, b):
    from concourse.bass_utils import run_bass_kernel_spmd

    x = np.asarray(x, dtype=np.float32)
    W = np.asarray(W, dtype=np.float32)
    b = np.asarray(b, dtype=np.float32)
    p = prepare(x, edge_index, W, b)
    nc = build_program(p)
    res = run_bass_kernel_spmd(nc, make_in_maps(p), core_ids=list(range(p.n_cores)))
    outs = [r["out"] for r in res.results]
    return _unshard(p, outs)


# revision 18
# speedup vs baseline: 1.0620x; 1.0620x over previous
"""GCN message-passing kernel for Trainium2 (8 NeuronCores, Bass/Tile).

out = coef * relu(C_U * D^-1/2 A~^T D^-1/2 (x W^T + b)),  A~ = A + I

Strategy (dst-sharded, fully static SPMD program, host-materialized
message stream):
- Core c owns a 12,500-node dst range. Host assigns dsts to 98
  windows of up to 128 members (LPT by in-degree for load balance),
  then lays every edge message (incl. self loops) out window-major as
  a dense bf16 stream xg[lane, col, :] = dis_src * x[src]  (the
  degree prescale and bf16 cast are done once on host; W commutes
  with aggregation so it is applied after the scatter).
- Device: pure sequential DMA of the xg stream (no gather); DVE
  builds one-hot [slot, 64] matrices from dst offsets (iota==doff,
  contiguous inner dim so the matmul rhs columns are contiguous);
  PE contracts msgs^T @ onehot into a shared PSUM bank per window
  PAIR ([128=D, 2x64 dst]); one stage-2 matmul applies W^T per pair;
  ACT fuses relu + coef*C_U*dis_dst scale; bf16 DMA out on the
  scalar HWDGE ring (host upcasts + unpermutes). All streaming in
  bf16 (strided-rhs and fp16 moving operands stall the PE).
"""

import sys
import types

import numpy as np


def _install_ntff_hook_bridge():
    """antenv.axon_hooks is missing from this image; bridge it so
    run_bass_kernel_spmd(trace=True) can profile. Harmless if unused."""
    if "antenv.axon_hooks" in sys.modules:
        return
    hooks = types.ModuleType("antenv.axon_hooks")
    hooks._HOOK = None

    def _get():
        if hooks._HOOK is None:
            try:
                from trn_agent_boot.trn_boot import _ntff_profile_via_ctypes

                hooks._HOOK = _ntff_profile_via_ctypes("/opt/axon/libaxon_pjrt.so")
            except Exception:
                hooks._HOOK = None
        return hooks._HOOK

    hooks.get_axon_ntff_profile_hook = _get
    hooks.set_axon_ntff_profile_hook = lambda h: setattr(hooks, "_HOOK", h)
    sys.modules["antenv.axon_hooks"] = hooks


_install_ntff_hook_bridge()

C_SIGMA = 2.0
C_U = 1.0
W_WIN = 64  # dst window width (one-hot width)
N_CORES = 8
GMAX_COLS = 114  # max slot-columns per double-buffered group (~3.7MB DMA)


def _ceil(a, b):
    return (a + b - 1) // b


class _Prep:
    """Host-side sharding/preprocessing result."""


def _pack_windows(indeg, nwin, cap):
    """LPT: assign items (sorted by weight desc) to nwin bins with
    member cap; then swap-rebalance toward max load <= ceil128(mean).
    Returns (win_of, pos_of, loads, counts)."""
    n = indeg.shape[0]
    order = np.argsort(-indeg, kind="stable")
    loads = np.zeros(nwin, dtype=np.int64)
    counts = np.zeros(nwin, dtype=np.int64)
    win_of = np.empty(n, dtype=np.int32)
    big = np.int64(1) << 50
    for d in order:
        w = int(np.argmin(loads))
        win_of[d] = w
        counts[w] += 1
        loads[w] += indeg[d]
        if counts[w] >= cap:
            loads[w] += big  # full -> never chosen again
    loads[loads >= big] -= big
    # swap rebalance: move load between windows via member swaps (keeps
    # member counts) until each window fits its per-window capacity.
    # Two-tier capacities concentrate the above-mean remainder in a few
    # overflow windows instead of pushing every window over a column
    # boundary.
    S = int(indeg.sum())
    base = _ceil(_ceil(S, nwin), 128) * 128
    caps = np.full(nwin, base, dtype=np.int64)
    base2 = base - 128
    excess2 = S - nwin * base2
    n_over = _ceil(max(excess2, 0), 128)
    if 0 < n_over < nwin:
        caps[:] = base2
        heavy = np.argsort(-loads)[:n_over]
        caps[heavy] = base2 + 128 * _ceil(_ceil(excess2, n_over), 128)
    for _ in range(3000):
        viol = loads - caps
        hi = int(np.argmax(viol))
        if viol[hi] <= 0:
            break
        lo = int(np.argmin(viol))
        mh = np.where(win_of == hi)[0]
        ml = np.where(win_of == lo)[0]
        da, db = indeg[mh], indeg[ml]
        diff = da[:, None] - db[None, :]
        ok = diff > 0
        if not ok.any():
            break
        head = -viol[lo]
        tgt = min(viol[hi], head) if head > 0 else 1
        score = np.where(ok, np.abs(diff - tgt), np.int64(1) << 40)
        i, j = np.unravel_index(np.argmin(score), score.shape)
        a, b = mh[i], ml[j]
        win_of[a], win_of[b] = lo, hi
        delta = indeg[a] - indeg[b]
        loads[hi] -= delta
        loads[lo] += delta
    # positions: recompute per window
    pos_of = np.empty(n, dtype=np.int32)
    counts = np.zeros(nwin, dtype=np.int64)
    for d in range(n):
        w = win_of[d]
        pos_of[d] = counts[w]
        counts[w] += 1
    return win_of, pos_of, loads, counts


def prepare(x, edge_index, W, b, n_cores=N_CORES, w_win=W_WIN):
    import ml_dtypes

    f16 = ml_dtypes.bfloat16
    N, D = x.shape
    assert N % n_cores == 0
    npc = N // n_cores
    nwin = _ceil(npc, w_win)
    if nwin % 2:
        nwin += 1  # window pairing needs an even count

    src = np.asarray(edge_index[0], dtype=np.int64)
    dst = np.asarray(edge_index[1], dtype=np.int64)
    deg = np.bincount(src, minlength=N).astype(np.float32) + 1.0
    dis = deg ** -0.5  # float32

    p = _Prep()
    p.N, p.D, p.npc, p.nwin = N, D, npc, nwin
    p.n_cores, p.w_win = n_cores, w_win
    p.coef = np.sqrt(C_SIGMA / D).astype(np.float32)
    p.xs16 = (dis[:, None] * np.asarray(x, dtype=np.float32)).astype(f16)

    core_of = dst // npc
    dstloc = (dst - core_of * npc).astype(np.int64)

    # --- per-core window assignment (LPT by in-degree incl self loop)
    p.win_of = np.empty((n_cores, npc), dtype=np.int32)
    p.pos_of = np.empty((n_cores, npc), dtype=np.int32)
    loads_all = np.empty((n_cores, nwin), dtype=np.int64)
    for c in range(n_cores):
        indeg = np.bincount(dstloc[core_of == c], minlength=npc) + 1
        win_of, pos_of, loads, _ = _pack_windows(indeg, nwin, w_win)
        # canonical order: windows sorted by load desc so the static
        # per-window column schedule (max over cores) stays tight
        order = np.argsort(-loads, kind="stable")
        rank = np.empty(nwin, dtype=np.int32)
        rank[order] = np.arange(nwin, dtype=np.int32)
        p.win_of[c] = rank[win_of]
        p.pos_of[c] = pos_of
        loads_all[c] = loads[order]

    # --- static per-window column schedule (shared by all cores)
    p.wcols = np.maximum(1, _ceil(loads_all.max(axis=0), 128)).astype(np.int64)
    p.colstart = np.concatenate([[0], np.cumsum(p.wcols)[:-1]])
    p.totcols = int(p.wcols.sum())

    # --- groups of consecutive windows for double-buffered streaming
    # (even window counts preferred so stage-1 chains interleave in pairs)
    p.groups = []  # (w0, nwins, colstart, ncols)
    w0 = 0
    while w0 < nwin:
        w1 = w0
        cols = 0
        gcap = 4 * p.wcols[0] if w0 == 0 else GMAX_COLS  # small head group
        while w1 < nwin and cols + p.wcols[w1] <= gcap:
            cols += int(p.wcols[w1])
            w1 += 1
        if (w1 - w0) > 1 and (w1 - w0) % 2 == 1 and w1 < nwin:
            cols -= int(p.wcols[w1 - 1])
            w1 -= 1
        p.groups.append((w0, w1 - w0, int(p.colstart[w0]), cols))
        w0 = w1
    p.gmax_cols = max(g[3] for g in p.groups)
    p.gmax_wins = max(g[1] for g in p.groups)

    # --- per-core edge slot fill
    p.xg = []
    p.doff = []
    p.sd = []
    p.memb = []
    for c in range(n_cores):
        m = core_of == c
        e_src = np.concatenate([src[m], np.arange(npc, dtype=np.int64) + c * npc])
        e_dl = np.concatenate([dstloc[m], np.arange(npc, dtype=np.int64)])
        e_w = p.win_of[c][e_dl]
        e_off = p.pos_of[c][e_dl]
        o = np.argsort(e_w, kind="stable")
        e_src, e_w, e_off = e_src[o], e_w[o], e_off[o]
        wcnt = np.bincount(e_w, minlength=nwin)
        starts = np.concatenate([[0], np.cumsum(wcnt)[:-1]])
        rank = np.arange(len(e_w)) - starts[e_w]
        lane = rank % 128
        col = p.colstart[e_w] + rank // 128
        assert (rank < p.wcols[e_w] * 128).all()

        xg = np.zeros((128, p.totcols, D), dtype=f16)
        xg[lane, col] = p.xs16[e_src]
        doff = np.full((128, p.totcols), -1.0, dtype=f16)
        doff[lane, col] = e_off
        p.xg.append(xg)
        p.doff.append(doff)

        sd = np.zeros((w_win, nwin), dtype=np.float32)
        sd[p.pos_of[c], p.win_of[c]] = p.coef * C_U * dis[c * npc : (c + 1) * npc]
        # pair windows (2q, 2q+1) stacked on 128 partitions
        sdp = np.zeros((2 * w_win, nwin // 2), dtype=np.float32)
        sdp[:w_win] = sd[:, 0::2]
        sdp[w_win:] = sd[:, 1::2]
        p.sd.append(sdp)
        memb = -np.ones(nwin * w_win, dtype=np.int64)
        memb[p.win_of[c].astype(np.int64) * w_win + p.pos_of[c]] = np.arange(npc)
        p.memb.append(memb)

    # iota row constant [128, 1, w_win]: value j at (p, 0, j)
    io = np.broadcast_to(
        np.arange(w_win, dtype=np.float32)[None, None, :], (128, 1, w_win)
    )
    p.iota = np.ascontiguousarray(io.astype(f16))
    p.WT = np.ascontiguousarray(np.asarray(W, dtype=np.float32).T)
    p.b = np.asarray(b, dtype=np.float32)
    p.bias_nonzero = bool(np.any(p.b != 0))
    if p.bias_nonzero:
        # S_d = sum over incoming edges (incl self) of dis_src, laid out
        # as a row [1, nwin*w_win]: element w*w_win + i = window w, row i
        sb = np.zeros((n_cores, nwin * w_win), dtype=np.float32)
        slot = (
            p.win_of[core_of, dstloc].astype(np.int64) * w_win
            + p.pos_of[core_of, dstloc]
        )
        np.add.at(sb, (core_of, slot), dis[src])
        sl = np.arange(N, dtype=np.int64)
        slot_self = (
            p.win_of[core_of_sl := sl // npc, sl % npc].astype(np.int64) * w_win
            + p.pos_of[core_of_sl, sl % npc]
        )
        np.add.at(sb, (core_of_sl, slot_self), dis[sl])
        p.sb = sb.reshape(n_cores, 1, nwin * w_win)
    return p


def build_program(p):
    import concourse.bacc as bacc
    import concourse.mybir as mybir
    import concourse.tile as tile

    f32, f16i = mybir.dt.float32, mybir.dt.bfloat16
    D, nwin, w_win = p.D, p.nwin, p.w_win

    nc = bacc.Bacc("TRN2", target_bir_lowering=False, debug=False)
    xg_d = nc.dram_tensor("xg", [128, p.totcols, D], f16i, kind="ExternalInput")
    wt_d = nc.dram_tensor("wt", [D, D], f32, kind="ExternalInput")
    iota_d = nc.dram_tensor("iota", [128, 1, w_win], f16i, kind="ExternalInput")
    doff_d = nc.dram_tensor("doff", [128, p.totcols], f16i, kind="ExternalInput")
    sd_d = nc.dram_tensor("sd", [2 * w_win, nwin // 2], f32, kind="ExternalInput")
    if p.bias_nonzero:
        sb_d = nc.dram_tensor("sb", [1, nwin * w_win], f32, kind="ExternalInput")
        b_d = nc.dram_tensor("b", [1, D], f32, kind="ExternalInput")
    # partition-major layout: out[r*64+i, q, :] = window 2q+r, row i
    out_d = nc.dram_tensor("out", [2 * w_win, nwin // 2, D], f16i, kind="ExternalOutput")

    with tile.TileContext(nc) as tc:
        with (
            tc.tile_pool(name="const", bufs=1) as constp,
            tc.tile_pool(name="msgs", bufs=3) as msgsp,
            tc.tile_pool(name="vh", bufs=3) as vhp,
            tc.tile_pool(name="aggx", bufs=3) as aggxp,
            tc.tile_pool(name="outsb", bufs=2) as outp,
            tc.tile_pool(name="ps1", bufs=4, space="PSUM") as ps1p,
            tc.tile_pool(name="ps2", bufs=2, space="PSUM") as ps2p,
        ):
            # constants
            wt32 = constp.tile([D, D], f32, tag="wt32")
            nc.sync.dma_start(wt32[:], wt_d[:])
            wt16 = constp.tile([D, D], f16i, tag="wt16")
            nc.scalar.copy(wt16[:], wt32[:])
            iota_sb = constp.tile([128, 1, w_win], f16i, tag="iota")
            nc.sync.dma_start(iota_sb[:], iota_d[:])
            sd_sb = constp.tile([2 * w_win, nwin // 2], f32, tag="sd")
            nc.sync.dma_start(sd_sb[:], sd_d[:])
            doff_sb = constp.tile([128, p.totcols], f16i, tag="doff")
            nc.sync.dma_start(doff_sb[:], doff_d[:])
            if p.bias_nonzero:
                sb_sb = constp.tile([1, nwin * w_win], f32, tag="sb")
                nc.sync.dma_start(sb_sb[:], sb_d[:])
                b32 = constp.tile([1, D], f32, tag="b32")
                nc.sync.dma_start(b32[:], b_d[:])
                b16 = constp.tile([1, D], f16i, tag="b16")
                nc.scalar.copy(b16[:], b32[:])
                sbrow16 = constp.tile([1, nwin * w_win], f16i, tag="sbw16")
                nc.scalar.copy(sbrow16[:], sb_sb[:])

            for w0, gs, c0, gcols in p.groups:
                ms = msgsp.tile([128, gcols, D], f16i, tag="ms")
                nc.sync.dma_start(ms[:], xg_d[:, c0 : c0 + gcols, :])
                vt = vhp.tile([128, gcols, w_win], f16i, tag="vh")
                nc.vector.tensor_tensor(
                    vt[:],
                    iota_sb[:].broadcast_to([128, gcols, w_win]),
                    doff_sb[:, c0 : c0 + gcols]
                    .rearrange("p (c o) -> p c o", o=1)
                    .broadcast_to([128, gcols, w_win]),
                    mybir.AluOpType.is_equal,
                )
                out_sb = outp.tile([2 * w_win, gs // 2, D], f16i, tag="out")
                # windows processed in pairs sharing one PSUM bank: chain r
                # accumulates into ps1[:, r*64:(r+1)*64]; one ag copy,
                # stage-2 matmul and relu+scale activation per pair
                for pl0 in range(0, gs, 2):
                    ps1 = ps1p.tile([D, 2 * w_win], f32, tag="ps1")
                    for r in (0, 1):
                        w = w0 + pl0 + r
                        cw = int(p.wcols[w])
                        cl = int(p.colstart[w]) - c0
                        for k in range(cw):
                            nc.tensor.matmul(
                                ps1[:, r * w_win : (r + 1) * w_win],
                                ms[:, cl + k, :],
                                vt[:, cl + k, :],
                                start=(k == 0),
                                stop=(k == cw - 1),
                            )
                    ag = aggxp.tile([D, 2 * w_win], f16i, tag="ag")
                    nc.scalar.copy(ag[:], ps1[:])
                    ps2 = ps2p.tile([2 * w_win, D], f32, tag="ps2")
                    nc.tensor.matmul(
                        ps2[:, :],
                        ag[:, :],
                        wt16[:, :],
                        start=True,
                        stop=not p.bias_nonzero,
                    )
                    q = (w0 + pl0) // 2
                    if p.bias_nonzero:
                        nc.tensor.matmul(
                            ps2[:, :],
                            sbrow16[:, q * 2 * w_win : (q + 1) * 2 * w_win],
                            b16[:, :],
                            start=False,
                            stop=True,
                        )
                    nc.scalar.activation(
                        out_sb[:, pl0 // 2, :],
                        ps2[:, :],
                        mybir.ActivationFunctionType.Relu,
                        scale=sd_sb[:, q : q + 1],
                    )
                nc.scalar.dma_start(out_d[:, w0 // 2 : (w0 + gs) // 2, :], out_sb[:])
    nc.compile()
    return nc


def make_in_maps(p):
    in_maps = []
    for c in range(p.n_cores):
        m = {
            "xg": p.xg[c],
            "wt": p.WT,
            "iota": p.iota,
            "doff": p.doff[c],
            "sd": p.sd[c],
        }
        if p.bias_nonzero:
            m["sb"] = p.sb[c]
            m["b"] = p.b.reshape(1, -1)
        in_maps.append(m)
    return in_maps


def _unshard(p, outs):
    N, D = p.N, p.D
    res = np.empty((N, D), dtype=np.float32)
    for c in range(p.n_cores):
        # [2*w_win, nwin//2, D]: window 2q+r at rows [r*w_win:(r+1)*w_win]
        o = np.asarray(outs[c]).astype(np.float32)
        o = (
            o.reshape(2, p.w_win, p.nwin // 2, D)
            .transpose(2, 0, 1, 3)
            .reshape(p.nwin * p.w_win, D)
        )
        memb = p.memb[c]
        real = memb >= 0
        res[c * p.npc + memb[real]] = o[real]
    return res


def kernel(x, edge_index, W, b):
    from concourse.bass_utils import run_bass_kernel_spmd

    x = np.asarray(x, dtype=np.float32)
    W = np.asarray(W, dtype=np.float32)
    b = np.asarray(b, dtype=np.float32)
    p = prepare(x, edge_index, W, b)
    nc = build_program(p)
    res = run_bass_kernel_spmd(nc, make_in_maps(p), core_ids=list(range(p.n_cores)))
    outs = [r["out"] for r in res.results]
    return _unshard(p, outs)
